# revision 14
# baseline (speedup 1.0000x reference)
"""BoundaryAttentionModule Trainium2 kernel — segment decomposition, fp8 DR.

Shapes (hardcoded): b=4, c=256, h=w=64 (HW=4096), boundary 128x128,
mid=64, out_ch=256. 8 cores: core = (batch bi = core//2, key-half kh = core%2).

The energy E[k,j] = relu(kw1f*t_k + beta)^T G[:,j] depends on key k only
through the scalar t_k = bm[k], piecewise-linear in t with <=64 relu
breakpoints.  S=64 pseudo-segments x 2 edge anchors -> 128 rows:

  E2   = (ME2 @ M) @ u          [128, 4096]   (one matmul from u)
  Fcat = exp(E2 / SC)           [128, 4096]   (ME2M shipped x SC for fp8)
  exp(E[k,:]) ~= wl_k*Fcat[2s_k,:] + wh_k*Fcat[2s_k+1,:]   (host-known wl/wh)
  s_k  = wl_k*SA[2s_k] + wh_k*SA[2s_k+1],  SA = Fcat row sums
  P    = W^T @ Fcat,  W[r,c] = sum_k wmask[k,r]/s_k * Vt[k,c]

u / ME2M / value_w^T travel in fp8e4 (ME2M, vwt pre-scaled x64 to clear the
e4m3 denormal floor; host divides gamma by 64).  Channel axis is pairwise
interleaved so E2 and Vt run in DoubleRow mode (256-deep contraction per
pass).  Masks/everything else bf16, PSUM f32.

host: out[bi] = gamma/SC * (P[2bi] + P[2bi+1]) + u[bi]
"""

import numpy as np

B, C, HW = 4, 256, 4096
KH = HW // 2          # 2048 keys per core
NK = KH // 128        # 16 k-tiles
S = 64                # pseudo-segments
NR = 2 * S            # 128 anchor rows
SC = 64.0             # fp8 pre-scale on ME2M / vwt

TRACE = False
TRACE_CORES = None
LAST_RESULTS = None

_BUILT = None


def _build():
    import concourse.bass as bass
    import concourse.tile as tile
    from concourse import bacc, mybir

    f32 = mybir.dt.float32
    bf16 = mybir.dt.bfloat16
    fp8 = mybir.dt.float8e4
    AF = mybir.ActivationFunctionType
    AX = mybir.AxisListType
    ALU = mybir.AluOpType
    DR = mybir.MatmulPerfMode.DoubleRow

    nc = bacc.Bacc(
        "TRN2",
        target_bir_lowering=False,
        debug=False,
        enable_asserts=False,
        num_devices=8,
    )

    # u chunks: row block ci*128+p, col i*1024+x  <->  u8[2p+i, ci*1024+x]
    u_in = nc.dram_tensor("u_in", [512, 2048], fp8, kind="ExternalInput").ap()
    # wpack: cols 0:256 me2-dr (i*128+r), cols 256:768 vwt-dr (i*256+c)
    wpack_in = nc.dram_tensor("wpack_in", [128, 768], fp8, kind="ExternalInput").ap()
    wmask_in = nc.dram_tensor("wmask_in", [128, NK * NR], bf16, kind="ExternalInput").ap()
    gw_in = nc.dram_tensor("gw_in", [NR, KH], bf16, kind="ExternalInput").ap()
    out_d = nc.dram_tensor("outp", [C, HW], bf16, kind="ExternalOutput").ap()

    with tile.TileContext(nc) as tc:
        with (
            tc.tile_pool(name="sb", bufs=1) as sb,
            tc.tile_pool(name="ps", bufs=1, space="PSUM") as ps,
        ):
            # ---- input DMAs: each ring's most-urgent DMA first (rings
            # round-robin among queued DMAs, so bulk loads sharing a ring
            # delay every member's completion to the drain time) ----
            uc = []
            for ci in range(4):
                t = sb.tile([128, 2048], fp8, tag=f"uc{ci}", name=f"uc{ci}")
                uc.append(t)
            wpk = sb.tile([128, 768], fp8, tag="wpk", name="wpk")
            wmask = sb.tile([128, NK * NR], bf16, tag="wmask", name="wmask")
            gw = sb.tile([NR, KH], bf16, tag="gw", name="gw")
            nc.sync.dma_start(uc[0][:], u_in[0:128, :])
            nc.sync.dma_start(wmask[:], wmask_in[:, :])
            nc.scalar.dma_start(wpk[:], wpack_in[:, :])
            nc.scalar.dma_start(uc[1][:], u_in[128:256, :])
            nc.scalar.dma_start(uc[3][:], u_in[384:512, :])
            nc.gpsimd.dma_start(uc[2][:], u_in[256:384, :])
            nc.gpsimd.dma_start(gw[:], gw_in[:, :])

            # dummy exp to prepay the ACT table load (after the scalar-queue
            # DMA posts so it doesn't delay them)
            dsrc = sb.tile([128, 1], bf16, tag="dsrc", name="dsrc")
            ddst = sb.tile([128, 1], bf16, tag="ddst", name="ddst")
            nc.vector.memset(dsrc[:], 0.0)
            nc.scalar.activation(ddst[:], dsrc[:], AF.Exp)

            me2v = wpk[:, 0:256].rearrange("p (i r) -> p i r", i=2)
            vwtv = wpk[:, 256:768].rearrange("p (i c) -> p i c", i=2)

            Fcat = sb.tile([128, HW], bf16, tag="Fcat", name="Fcat")
            SAp = sb.tile([128, 4], f32, tag="SAp", name="SAp")
            SAb4 = sb.tile([128, 4], bf16, tag="SAb4", name="SAb4")
            rs = sb.tile([128, NK], f32, tag="rs", name="rs")
            rinv = sb.tile([128, NK], f32, tag="rinv", name="rinv")
            vtb = sb.tile([128, NK * C], bf16, tag="vtb", name="vtb")
            wsc = sb.tile([128, NK * NR], bf16, tag="wsc", name="wsc")
            Wsb = sb.tile([128, C], bf16, tag="Wsb", name="Wsb")
            osb0 = sb.tile([128, HW], bf16, tag="osb0", name="osb0")
            osb1 = sb.tile([128, HW], bf16, tag="osb1", name="osb1")

            # ---- phase 1: E2 + exp (all j), Vt (keys = cols 0:2048) ----
            for ci in range(4):
                ucv = uc[ci].rearrange("p (i x) -> p i x", i=2)
                e2 = ps.tile([128, 1024], f32, tag="big", bufs=2, name=f"e2_{ci}")
                for q in range(2):
                    nc.tensor.matmul(
                        e2[:, q * 512 : (q + 1) * 512], me2v,
                        ucv[:, :, q * 512 : (q + 1) * 512],
                        start=True, stop=True, perf_mode=DR,
                    )
                nc.scalar.activation(
                    Fcat[:, ci * 1024 : (ci + 1) * 1024], e2[:, 0:1024], AF.Exp,
                    scale=1.0 / SC, accum_out=SAp[:, ci : ci + 1],
                )
                if ci < 2:
                    for g in range(4):
                        vt = ps.tile([128, 512], f32, tag="vt", bufs=2,
                                     name=f"vt_{ci}_{g}")
                        for t2 in range(2):
                            kt = ci * 8 + g * 2 + t2
                            lx = (kt % 8) * 128
                            nc.tensor.matmul(
                                vt[:, t2 * 256 : (t2 + 1) * 256],
                                ucv[:, :, lx : lx + 128], vwtv,
                                start=True, stop=True, perf_mode=DR,
                            )
                        kt0 = ci * 8 + g * 2
                        nc.vector.tensor_copy(
                            vtb[:, kt0 * 256 : (kt0 + 2) * 256], vt[:]
                        )

            # ---- softmax denominators: SA -> gather -> 1/s ----
            nc.vector.reduce_sum(rs[:, 0:1], SAp[:, 0:4], axis=AX.X)
            nc.vector.tensor_copy(SAb4[:, 0:1], rs[:, 0:1])
            sg = ps.tile([128, NK], f32, tag="sg", name="sg")
            for kt in range(NK):
                nc.tensor.matmul(
                    sg[:, kt : kt + 1], gw[:, kt * 128 : (kt + 1) * 128],
                    SAb4[:, 0:1],
                    start=True, stop=True,
                )
            nc.vector.reciprocal(rinv[:], sg[:])

            # ---- W = (wmask/s)^T @ Vt ----
            wps = ps.tile([128, C], f32, tag="w", name="wps")
            for kt in range(NK):
                nc.vector.tensor_scalar(
                    wsc[:, kt * NR : (kt + 1) * NR],
                    wmask[:, kt * NR : (kt + 1) * NR],
                    rinv[:, kt : kt + 1], None, op0=ALU.mult,
                )
                nc.tensor.matmul(
                    wps[:], wsc[:, kt * NR : (kt + 1) * NR],
                    vtb[:, kt * C : (kt + 1) * C],
                    start=(kt == 0), stop=(kt == NK - 1),
                )
            nc.scalar.copy(Wsb[:], wps[:])

            # ---- P = W^T @ Fcat -> bf16 -> DRAM ----
            # 512-wide copies on both engines in parallel keep the PSUM
            # double-buffer from stalling the matmul stream.
            for ct in range(2):
                osb = osb0 if ct == 0 else osb1
                for ci in range(4):
                    pp = ps.tile([128, 1024], f32, tag="big", bufs=2,
                                 name=f"pp_{ct}_{ci}")
                    for q in range(2):
                        sl = slice(ci * 1024 + q * 512, ci * 1024 + (q + 1) * 512)
                        nc.tensor.matmul(
                            pp[:, q * 512 : (q + 1) * 512],
                            Wsb[:, ct * 128 : (ct + 1) * 128], Fcat[:, sl],
                            start=True, stop=True,
                        )
                    sl = slice(ci * 1024, (ci + 1) * 1024)
                    if (ct * 4 + ci) % 2 == 0:
                        nc.scalar.copy(osb[:, sl], pp[:])
                    else:
                        nc.vector.tensor_copy(osb[:, sl], pp[:])
                    qeng = nc.sync if ct == 0 else nc.gpsimd
                    qeng.dma_start(
                        out_d[ct * 128 : (ct + 1) * 128, sl], osb[:, sl]
                    )

    nc.compile()
    return nc


def _get_built():
    global _BUILT
    if _BUILT is None:
        _BUILT = _build()
    return _BUILT


def _prep_global(boundary_map, key_w1, bn_scale, bn_bias, bn_mean, bn_var,
                 key_w2, query_w, value_w):
    """Segment geometry + weight folding (all float64 host math)."""
    b = boundary_map.shape[0]
    h = 64
    H0 = boundary_map.shape[2]
    idx = (np.arange(h) * H0) // h
    bm = boundary_map[:, 0][:, idx][:, :, idx].reshape(b, HW).astype(np.float64)

    inv = bn_scale.astype(np.float64) / np.sqrt(bn_var.astype(np.float64) + 1e-5)
    beta = bn_bias.astype(np.float64) - bn_mean.astype(np.float64) * inv
    kw1f = key_w1[:, 0].astype(np.float64) * inv
    M = key_w2.astype(np.float64).T @ query_w.astype(np.float64)   # [64, 256]

    tall = bm.reshape(-1)
    lo, hi = tall.min(), tall.max()
    with np.errstate(divide="ignore", invalid="ignore"):
        tstar = np.where(np.abs(kw1f) > 1e-30, -beta / kw1f, np.inf)
    bps = np.sort(tstar[(tstar > lo) & (tstar < hi)])
    edges_true = np.concatenate([[lo], bps, [hi]])
    nseg = len(edges_true) - 1
    assert nseg <= S, f"too many relu segments: {nseg}"

    widths = np.diff(edges_true)
    cnt = np.histogram(tall, bins=edges_true)[0]
    score = widths * np.sqrt(cnt + 1)
    alloc = np.maximum(1, np.floor(score / score.sum() * S).astype(int))
    while alloc.sum() < S:
        alloc[np.argmax(score / alloc)] += 1
    while alloc.sum() > S:
        cand = np.where(alloc > 1)[0]
        alloc[cand[np.argmin((score / alloc)[cand])]] -= 1

    ps_lo = np.empty(S); ps_hi = np.empty(S); ps_true = np.empty(S, np.int64)
    si_out = 0
    for si in range(nseg):
        sub = np.linspace(edges_true[si], edges_true[si + 1], alloc[si] + 1)
        for k in range(alloc[si]):
            ps_lo[si_out] = sub[k]; ps_hi[si_out] = sub[k + 1]
            ps_true[si_out] = si
            si_out += 1
    assert si_out == S

    mids = 0.5 * (edges_true[:-1] + edges_true[1:])
    act = (kw1f[None, :] * mids[:, None] + beta[None, :]) > 0     # [nseg, 64]
    ME2 = np.zeros((NR, 64))
    for s in range(S):
        si = ps_true[s]
        for a, ta in enumerate((ps_lo[s], ps_hi[s])):
            ME2[2 * s + a] = ta * (act[si] * kw1f) + act[si] * beta
    me2m = ME2 @ M                                                # [128, 256]

    seg = np.clip(np.searchsorted(ps_hi, bm, side="left"), 0, S - 1)  # [b, HW]
    width = np.maximum(ps_hi[seg] - ps_lo[seg], 1e-12)
    wl = (ps_hi[seg] - bm) / width
    wh = 1.0 - wl
    return bm, me2m, seg, wl, wh


def _host_prep(boundary_map, uncertainty_map, key_w1, bn_scale, bn_bias,
               bn_mean, bn_var, key_w2, query_w, value_w):
    import ml_dtypes

    bf16 = ml_dtypes.bfloat16
    f8 = ml_dtypes.float8_e4m3
    b, c, h, w = uncertainty_map.shape
    _, me2m, seg, wl, wh = _prep_global(
        boundary_map, key_w1, bn_scale, bn_bias, bn_mean, bn_var,
        key_w2, query_w, value_w,
    )
    # wpack: me2-dr | vwt-dr, both pre-scaled by SC, channel-pair interleaved
    wpack = np.zeros((128, 768), np.float32)
    me2s = (me2m * SC).astype(np.float32)                          # [128r, 256c]
    vws = (value_w.T * SC).astype(np.float32)                      # [256c', 256c]
    for i in range(2):
        wpack[:, i * 128 : (i + 1) * 128] = me2s[:, i::2].T        # [p, r]
        wpack[:, 256 + i * 256 : 256 + (i + 1) * 256] = vws[i::2, :]
    wpack = wpack.astype(f8)

    in_maps = []
    for core in range(8):
        bi, kh = core // 2, core % 2
        u = uncertainty_map[bi].reshape(c, h * w).astype(np.float32)
        if kh == 1:
            u = np.concatenate([u[:, KH:], u[:, :KH]], axis=1)
        u8 = u.astype(f8)
        # chunk blocks: ub[ci*128+p, i*1024+x] = u8[2p+i, ci*1024+x]
        ub = np.ascontiguousarray(
            u8.reshape(128, 2, 4, 1024).transpose(2, 0, 1, 3).reshape(512, 2048)
        )
        ksl = slice(kh * KH, (kh + 1) * KH)
        sg_k = seg[bi, ksl]
        wmask = np.zeros((KH, NR), np.float32)
        kk = np.arange(KH)
        wmask[kk, 2 * sg_k] = wl[bi, ksl]
        wmask[kk, 2 * sg_k + 1] = wh[bi, ksl]
        wm_dev = np.ascontiguousarray(
            wmask.reshape(NK, 128, NR).transpose(1, 0, 2).reshape(128, NK * NR)
        ).astype(bf16)
        gw_dev = np.ascontiguousarray(
            wmask.reshape(NK, 128, NR).transpose(2, 0, 1).reshape(NR, KH)
        ).astype(bf16)
        in_maps.append({
            "u_in": ub,
            "wpack_in": wpack,
            "wmask_in": wm_dev,
            "gw_in": gw_dev,
        })
    return in_maps


def kernel(boundary_map, uncertainty_map, key_w1, bn_scale, bn_bias,
           bn_mean, bn_var, key_w2, query_w, value_w, gamma):
    global LAST_RESULTS
    from concourse.bass_utils import run_bass_kernel_spmd

    nc = _get_built()
    in_maps = _host_prep(
        np.asarray(boundary_map), np.asarray(uncertainty_map), np.asarray(key_w1),
        np.asarray(bn_scale), np.asarray(bn_bias), np.asarray(bn_mean),
        np.asarray(bn_var), np.asarray(key_w2), np.asarray(query_w),
        np.asarray(value_w),
    )
    kwargs = {}
    if TRACE:
        kwargs["trace"] = True
        if TRACE_CORES is not None:
            kwargs["trace_cores"] = TRACE_CORES
    res = run_bass_kernel_spmd(nc, in_maps, core_ids=list(range(8)), **kwargs)
    LAST_RESULTS = res

    b, c, h, w = uncertainty_map.shape
    g = np.float32(np.asarray(gamma).reshape(-1)[0] / SC)
    out = np.empty((b, c, h * w), np.float32)
    um = np.asarray(uncertainty_map)
    for bi in range(b):
        P0 = res.results[2 * bi]["outp"].astype(np.float32)
        P1 = res.results[2 * bi + 1]["outp"].astype(np.float32)
        P1 = np.concatenate([P1[:, KH:], P1[:, :KH]], axis=1)
        out[bi] = g * (P0 + P1) + um[bi].reshape(c, h * w)
    return out.reshape(b, c, h, w)


# revision 16
# speedup vs baseline: 1.1768x; 1.1768x over previous
"""BoundaryAttentionModule Trainium2 kernel — segment decomposition, fp8 DR.

Shapes (hardcoded): b=4, c=256, h=w=64 (HW=4096), boundary 128x128,
mid=64, out_ch=256. 8 cores: core = (batch bi = core//2, key-half kh = core%2).

The energy E[k,j] = relu(kw1f*t_k + beta)^T G[:,j] depends on key k only
through the scalar t_k = bm[k], piecewise-linear in t with <=64 relu
breakpoints.  S=64 pseudo-segments x 2 edge anchors -> 128 rows:

  E2   = (ME2 @ M) @ u          [128, 4096]   (one matmul from u)
  Fcat = exp(E2 / SC)           [128, 4096]   (ME2M shipped x SC for fp8)
  exp(E[k,:]) ~= wl_k*Fcat[2s_k,:] + wh_k*Fcat[2s_k+1,:]   (host-known wl/wh)
  s_k  = wl_k*SA[2s_k] + wh_k*SA[2s_k+1],  SA = Fcat row sums
  P    = W^T @ Fcat,  W[r,c] = sum_k wmask[k,r]/s_k * Vt[k,c]

u / ME2M / value_w^T travel in fp8e4 (ME2M, vwt pre-scaled x64 to clear the
e4m3 denormal floor; host divides gamma by 64).  Channel axis is pairwise
interleaved so E2 and Vt run in DoubleRow mode (256-deep contraction per
pass).  Masks/everything else bf16, PSUM f32.

host: out[bi] = gamma/SC * (P[2bi] + P[2bi+1]) + u[bi]
"""

import numpy as np

B, C, HW = 4, 256, 4096
KH = HW // 2          # 2048 keys per core
NK = KH // 128        # 16 k-tiles
S = 64                # pseudo-segments
NR = 2 * S            # 128 anchor rows
SC = 64.0             # fp8 pre-scale on ME2M / vwt

TRACE = False
TRACE_CORES = None
LAST_RESULTS = None

_BUILT = None


def _build():
    import concourse.bass as bass
    import concourse.tile as tile
    from concourse import bacc, mybir

    f32 = mybir.dt.float32
    bf16 = mybir.dt.bfloat16
    fp8 = mybir.dt.float8e4
    AF = mybir.ActivationFunctionType
    AX = mybir.AxisListType
    ALU = mybir.AluOpType
    DR = mybir.MatmulPerfMode.DoubleRow

    nc = bacc.Bacc(
        "TRN2",
        target_bir_lowering=False,
        debug=False,
        enable_asserts=False,
        num_devices=8,
    )

    # u chunks: row block ci*128+p, col i*1024+x  <->  u8[2p+i, ci*1024+x]
    u_in = nc.dram_tensor("u_in", [512, 2048], fp8, kind="ExternalInput").ap()
    # wpack: cols 0:256 me2-dr (i*128+r), cols 256:768 vwt-dr (i*256+c)
    wpack_in = nc.dram_tensor("wpack_in", [128, 768], fp8, kind="ExternalInput").ap()
    wmask_in = nc.dram_tensor("wmask_in", [128, NK * NR], bf16, kind="ExternalInput").ap()
    gw_in = nc.dram_tensor("gw_in", [NR, KH], bf16, kind="ExternalInput").ap()
    out_d = nc.dram_tensor("outp", [C, HW], bf16, kind="ExternalOutput").ap()

    with tile.TileContext(nc) as tc:
        with (
            tc.tile_pool(name="sb", bufs=1) as sb,
            tc.tile_pool(name="ps", bufs=1, space="PSUM") as ps,
        ):
            # ---- dummy exp to prepay the ACT table load ----
            dsrc = sb.tile([128, 1], bf16, tag="dsrc", name="dsrc")
            ddst = sb.tile([128, 1], bf16, tag="ddst", name="ddst")
            nc.vector.memset(dsrc[:], 0.0)
            nc.scalar.activation(ddst[:], dsrc[:], AF.Exp)

            # ---- input DMAs: each ring's most-urgent DMA first (rings
            # round-robin among queued DMAs, so bulk loads sharing a ring
            # delay every member's completion to the drain time) ----
            uc = []
            for ci in range(4):
                t = sb.tile([128, 2048], fp8, tag=f"uc{ci}", name=f"uc{ci}")
                uc.append(t)
            wpk = sb.tile([128, 768], fp8, tag="wpk", name="wpk")
            wmask = sb.tile([128, NK * NR], bf16, tag="wmask", name="wmask")
            gw = sb.tile([NR, KH], bf16, tag="gw", name="gw")
            nc.sync.dma_start(uc[0][:], u_in[0:128, :])
            nc.sync.dma_start(wmask[:], wmask_in[:, :])
            nc.scalar.dma_start(wpk[:], wpack_in[:, :])
            nc.scalar.dma_start(uc[1][:], u_in[128:256, :])
            nc.scalar.dma_start(uc[3][:], u_in[384:512, :])
            nc.gpsimd.dma_start(uc[2][:], u_in[256:384, :])
            nc.gpsimd.dma_start(gw[:], gw_in[:, :])

            me2v = wpk[:, 0:256].rearrange("p (i r) -> p i r", i=2)
            vwtv = wpk[:, 256:768].rearrange("p (i c) -> p i c", i=2)

            Fcat = sb.tile([128, HW], bf16, tag="Fcat", name="Fcat")
            SAp = sb.tile([128, 4], f32, tag="SAp", name="SAp")
            SAb4 = sb.tile([128, 4], bf16, tag="SAb4", name="SAb4")
            rs = sb.tile([128, NK], f32, tag="rs", name="rs")
            rinv = sb.tile([128, NK], f32, tag="rinv", name="rinv")
            vtb = sb.tile([128, NK * C], bf16, tag="vtb", name="vtb")
            wsc = sb.tile([128, NK * NR], bf16, tag="wsc", name="wsc")
            Wsb = sb.tile([128, C], bf16, tag="Wsb", name="Wsb")
            osb0 = sb.tile([128, HW], bf16, tag="osb0", name="osb0")
            osb1 = sb.tile([128, HW], bf16, tag="osb1", name="osb1")

            # ---- phase 1: E2 + exp (all j), Vt (keys = cols 0:2048) ----
            for ci in range(4):
                ucv = uc[ci].rearrange("p (i x) -> p i x", i=2)
                e2 = ps.tile([128, 1024], f32, tag="big", bufs=2, name=f"e2_{ci}")
                for q in range(2):
                    nc.tensor.matmul(
                        e2[:, q * 512 : (q + 1) * 512], me2v,
                        ucv[:, :, q * 512 : (q + 1) * 512],
                        start=True, stop=True, perf_mode=DR,
                    )
                nc.scalar.activation(
                    Fcat[:, ci * 1024 : (ci + 1) * 1024], e2[:, 0:1024], AF.Exp,
                    scale=1.0 / SC, accum_out=SAp[:, ci : ci + 1],
                )
                if ci < 2:
                    for g in range(4):
                        vt = ps.tile([128, 512], f32, tag="vt", bufs=2,
                                     name=f"vt_{ci}_{g}")
                        for t2 in range(2):
                            kt = ci * 8 + g * 2 + t2
                            lx = (kt % 8) * 128
                            nc.tensor.matmul(
                                vt[:, t2 * 256 : (t2 + 1) * 256],
                                ucv[:, :, lx : lx + 128], vwtv,
                                start=True, stop=True, perf_mode=DR,
                            )
                        kt0 = ci * 8 + g * 2
                        nc.vector.tensor_copy(
                            vtb[:, kt0 * 256 : (kt0 + 2) * 256], vt[:]
                        )

            # ---- softmax denominators: SA -> gather -> 1/s ----
            nc.vector.reduce_sum(rs[:, 0:1], SAp[:, 0:4], axis=AX.X)
            nc.vector.tensor_copy(SAb4[:, 0:1], rs[:, 0:1])
            sg = ps.tile([128, NK], f32, tag="sg", name="sg")
            for kt in range(NK):
                nc.tensor.matmul(
                    sg[:, kt : kt + 1], gw[:, kt * 128 : (kt + 1) * 128],
                    SAb4[:, 0:1],
                    start=True, stop=True,
                )
            nc.vector.reciprocal(rinv[:], sg[:])

            # ---- W = (wmask/s)^T @ Vt ----
            wps = ps.tile([128, C], f32, tag="w", name="wps")
            for kt in range(NK):
                nc.vector.tensor_scalar(
                    wsc[:, kt * NR : (kt + 1) * NR],
                    wmask[:, kt * NR : (kt + 1) * NR],
                    rinv[:, kt : kt + 1], None, op0=ALU.mult,
                )
                nc.tensor.matmul(
                    wps[:], wsc[:, kt * NR : (kt + 1) * NR],
                    vtb[:, kt * C : (kt + 1) * C],
                    start=(kt == 0), stop=(kt == NK - 1),
                )
            nc.scalar.copy(Wsb[:], wps[:])

            # ---- P = W^T @ Fcat -> bf16 -> DRAM ----
            # 512-wide copies on both engines in parallel keep the PSUM
            # double-buffer from stalling the matmul stream.
            for ct in range(2):
                osb = osb0 if ct == 0 else osb1
                for ci in range(4):
                    pp = ps.tile([128, 1024], f32, tag="big", bufs=2,
                                 name=f"pp_{ct}_{ci}")
                    for q in range(2):
                        sl = slice(ci * 1024 + q * 512, ci * 1024 + (q + 1) * 512)
                        nc.tensor.matmul(
                            pp[:, q * 512 : (q + 1) * 512],
                            Wsb[:, ct * 128 : (ct + 1) * 128], Fcat[:, sl],
                            start=True, stop=True,
                        )
                    sl = slice(ci * 1024, (ci + 1) * 1024)
                    if (ct * 4 + ci) % 2 == 0:
                        nc.scalar.copy(osb[:, sl], pp[:])
                    else:
                        nc.vector.tensor_copy(osb[:, sl], pp[:])
                    qeng = nc.sync if ct == 0 else nc.gpsimd
                    qeng.dma_start(
                        out_d[ct * 128 : (ct + 1) * 128, sl], osb[:, sl]
                    )

    nc.compile()
    return nc


def _get_built():
    global _BUILT
    if _BUILT is None:
        _BUILT = _build()
    return _BUILT


def _prep_global(boundary_map, key_w1, bn_scale, bn_bias, bn_mean, bn_var,
                 key_w2, query_w, value_w):
    """Segment geometry + weight folding (all float64 host math)."""
    b = boundary_map.shape[0]
    h = 64
    H0 = boundary_map.shape[2]
    idx = (np.arange(h) * H0) // h
    bm = boundary_map[:, 0][:, idx][:, :, idx].reshape(b, HW).astype(np.float64)

    inv = bn_scale.astype(np.float64) / np.sqrt(bn_var.astype(np.float64) + 1e-5)
    beta = bn_bias.astype(np.float64) - bn_mean.astype(np.float64) * inv
    kw1f = key_w1[:, 0].astype(np.float64) * inv
    M = key_w2.astype(np.float64).T @ query_w.astype(np.float64)   # [64, 256]

    tall = bm.reshape(-1)
    lo, hi = tall.min(), tall.max()
    with np.errstate(divide="ignore", invalid="ignore"):
        tstar = np.where(np.abs(kw1f) > 1e-30, -beta / kw1f, np.inf)
    bps = np.sort(tstar[(tstar > lo) & (tstar < hi)])
    edges_true = np.concatenate([[lo], bps, [hi]])
    nseg = len(edges_true) - 1
    assert nseg <= S, f"too many relu segments: {nseg}"

    widths = np.diff(edges_true)
    cnt = np.histogram(tall, bins=edges_true)[0]
    score = widths * np.sqrt(cnt + 1)
    alloc = np.maximum(1, np.floor(score / score.sum() * S).astype(int))
    while alloc.sum() < S:
        alloc[np.argmax(score / alloc)] += 1
    while alloc.sum() > S:
        cand = np.where(alloc > 1)[0]
        alloc[cand[np.argmin((score / alloc)[cand])]] -= 1

    ps_lo = np.empty(S); ps_hi = np.empty(S); ps_true = np.empty(S, np.int64)
    si_out = 0
    for si in range(nseg):
        sub = np.linspace(edges_true[si], edges_true[si + 1], alloc[si] + 1)
        for k in range(alloc[si]):
            ps_lo[si_out] = sub[k]; ps_hi[si_out] = sub[k + 1]
            ps_true[si_out] = si
            si_out += 1
    assert si_out == S

    mids = 0.5 * (edges_true[:-1] + edges_true[1:])
    act = (kw1f[None, :] * mids[:, None] + beta[None, :]) > 0     # [nseg, 64]
    ME2 = np.zeros((NR, 64))
    for s in range(S):
        si = ps_true[s]
        for a, ta in enumerate((ps_lo[s], ps_hi[s])):
            ME2[2 * s + a] = ta * (act[si] * kw1f) + act[si] * beta
    me2m = ME2 @ M                                                # [128, 256]

    seg = np.clip(np.searchsorted(ps_hi, bm, side="left"), 0, S - 1)  # [b, HW]
    width = np.maximum(ps_hi[seg] - ps_lo[seg], 1e-12)
    wl = (ps_hi[seg] - bm) / width
    wh = 1.0 - wl
    return bm, me2m, seg, wl, wh


def _host_prep(boundary_map, uncertainty_map, key_w1, bn_scale, bn_bias,
               bn_mean, bn_var, key_w2, query_w, value_w):
    import ml_dtypes

    bf16 = ml_dtypes.bfloat16
    f8 = ml_dtypes.float8_e4m3
    b, c, h, w = uncertainty_map.shape
    _, me2m, seg, wl, wh = _prep_global(
        boundary_map, key_w1, bn_scale, bn_bias, bn_mean, bn_var,
        key_w2, query_w, value_w,
    )
    # wpack: me2-dr | vwt-dr, both pre-scaled by SC, channel-pair interleaved
    wpack = np.zeros((128, 768), np.float32)
    me2s = (me2m * SC).astype(np.float32)                          # [128r, 256c]
    vws = (value_w.T * SC).astype(np.float32)                      # [256c', 256c]
    for i in range(2):
        wpack[:, i * 128 : (i + 1) * 128] = me2s[:, i::2].T        # [p, r]
        wpack[:, 256 + i * 256 : 256 + (i + 1) * 256] = vws[i::2, :]
    wpack = wpack.astype(f8)

    in_maps = []
    for core in range(8):
        bi, kh = core // 2, core % 2
        u = uncertainty_map[bi].reshape(c, h * w).astype(np.float32)
        if kh == 1:
            u = np.concatenate([u[:, KH:], u[:, :KH]], axis=1)
        u8 = u.astype(f8)
        # chunk blocks: ub[ci*128+p, i*1024+x] = u8[2p+i, ci*1024+x]
        ub = np.ascontiguousarray(
            u8.reshape(128, 2, 4, 1024).transpose(2, 0, 1, 3).reshape(512, 2048)
        )
        ksl = slice(kh * KH, (kh + 1) * KH)
        sg_k = seg[bi, ksl]
        wmask = np.zeros((KH, NR), np.float32)
        kk = np.arange(KH)
        wmask[kk, 2 * sg_k] = wl[bi, ksl]
        wmask[kk, 2 * sg_k + 1] = wh[bi, ksl]
        wm_dev = np.ascontiguousarray(
            wmask.reshape(NK, 128, NR).transpose(1, 0, 2).reshape(128, NK * NR)
        ).astype(bf16)
        gw_dev = np.ascontiguousarray(
            wmask.reshape(NK, 128, NR).transpose(2, 0, 1).reshape(NR, KH)
        ).astype(bf16)
        in_maps.append({
            "u_in": ub,
            "wpack_in": wpack,
            "wmask_in": wm_dev,
            "gw_in": gw_dev,
        })
    return in_maps


def kernel(boundary_map, uncertainty_map, key_w1, bn_scale, bn_bias,
           bn_mean, bn_var, key_w2, query_w, value_w, gamma):
    global LAST_RESULTS
    from concourse.bass_utils import run_bass_kernel_spmd

    nc = _get_built()
    in_maps = _host_prep(
        np.asarray(boundary_map), np.asarray(uncertainty_map), np.asarray(key_w1),
        np.asarray(bn_scale), np.asarray(bn_bias), np.asarray(bn_mean),
        np.asarray(bn_var), np.asarray(key_w2), np.asarray(query_w),
        np.asarray(value_w),
    )
    kwargs = {}
    if TRACE:
        kwargs["trace"] = True
        if TRACE_CORES is not None:
            kwargs["trace_cores"] = TRACE_CORES
    res = run_bass_kernel_spmd(nc, in_maps, core_ids=list(range(8)), **kwargs)
    LAST_RESULTS = res

    b, c, h, w = uncertainty_map.shape
    g = np.float32(np.asarray(gamma).reshape(-1)[0] / SC)
    out = np.empty((b, c, h * w), np.float32)
    um = np.asarray(uncertainty_map)
    for bi in range(b):
        P0 = res.results[2 * bi]["outp"].astype(np.float32)
        P1 = res.results[2 * bi + 1]["outp"].astype(np.float32)
        P1 = np.concatenate([P1[:, KH:], P1[:, :KH]], axis=1)
        out[bi] = g * (P0 + P1) + um[bi].reshape(c, h * w)
    return out.reshape(b, c, h, w)


# revision 18
# speedup vs baseline: 1.2388x; 1.0528x over previous
"""BoundaryAttentionModule Trainium2 kernel — segment decomposition, fp8 DR.

Shapes (hardcoded): b=4, c=256, h=w=64 (HW=4096), boundary 128x128,
mid=64, out_ch=256. 8 cores: core = (batch bi = core//2, key-half kh = core%2).

The energy E[k,j] = relu(kw1f*t_k + beta)^T G[:,j] depends on key k only
through the scalar t_k = bm[k], piecewise-linear in t with <=64 relu
breakpoints.  S=64 pseudo-segments x 2 edge anchors -> 128 rows:

  E2   = (ME2 @ M) @ u          [128, 4096]   (one matmul from u)
  Fcat = exp(E2 / SC)           [128, 4096]   (ME2M shipped x SC for fp8)
  exp(E[k,:]) ~= wl_k*Fcat[2s_k,:] + wh_k*Fcat[2s_k+1,:]   (host-known wl/wh)
  s_k  = wl_k*SA[2s_k] + wh_k*SA[2s_k+1],  SA = Fcat row sums
  P    = W^T @ Fcat,  W[r,c] = sum_k wmask[k,r]/s_k * Vt[k,c]

u / ME2M / value_w^T travel in fp8e4 (ME2M, vwt pre-scaled x64 to clear the
e4m3 denormal floor; host divides gamma by 64).  Channel axis is pairwise
interleaved so E2 and Vt run in DoubleRow mode (256-deep contraction per
pass).  Masks/everything else bf16, PSUM f32.

host: out[bi] = gamma/SC * (P[2bi] + P[2bi+1]) + u[bi]
"""

import numpy as np

B, C, HW = 4, 256, 4096
KH = HW // 2          # 2048 keys per core
NK = KH // 128        # 16 k-tiles
S = 64                # pseudo-segments
NR = 2 * S            # 128 anchor rows
SC = 64.0             # fp8 pre-scale on ME2M / vwt

TRACE = False
TRACE_CORES = None
LAST_RESULTS = None

_BUILT = None


def _build():
    import concourse.bass as bass
    import concourse.tile as tile
    from concourse import bacc, mybir

    f32 = mybir.dt.float32
    bf16 = mybir.dt.bfloat16
    fp8 = mybir.dt.float8e4
    AF = mybir.ActivationFunctionType
    AX = mybir.AxisListType
    ALU = mybir.AluOpType
    DR = mybir.MatmulPerfMode.DoubleRow

    nc = bacc.Bacc(
        "TRN2",
        target_bir_lowering=False,
        debug=False,
        enable_asserts=False,
        num_devices=8,
    )

    # u chunks: row block ci*128+p, col i*1024+x  <->  u8[2p+i, ci*1024+x]
    u_in = nc.dram_tensor("u_in", [512, 2048], fp8, kind="ExternalInput").ap()
    # wpack: cols 0:256 me2-dr (i*128+r), cols 256:768 vwt-dr (i*256+c)
    wpack_in = nc.dram_tensor("wpack_in", [128, 768], fp8, kind="ExternalInput").ap()
    wmask_in = nc.dram_tensor("wmask_in", [128, NK * NR], bf16, kind="ExternalInput").ap()
    gw_in = nc.dram_tensor("gw_in", [NR, KH], bf16, kind="ExternalInput").ap()
    out_d = nc.dram_tensor("outp", [C, HW], bf16, kind="ExternalOutput").ap()

    with tile.TileContext(nc) as tc:
        with (
            tc.tile_pool(name="sb", bufs=1) as sb,
            tc.tile_pool(name="ps", bufs=1, space="PSUM") as ps,
        ):
            # ---- dummy exp to prepay the ACT table load ----
            dsrc = sb.tile([128, 1], bf16, tag="dsrc", name="dsrc")
            ddst = sb.tile([128, 1], bf16, tag="ddst", name="ddst")
            nc.vector.memset(dsrc[:], 0.0)
            nc.scalar.activation(ddst[:], dsrc[:], AF.Exp)

            # ---- input DMAs: each ring's most-urgent DMA first (rings
            # round-robin among queued DMAs, so bulk loads sharing a ring
            # delay every member's completion to the drain time) ----
            uc = []
            for ci in range(4):
                t = sb.tile([128, 2048], fp8, tag=f"uc{ci}", name=f"uc{ci}")
                uc.append(t)
            wpk = sb.tile([128, 768], fp8, tag="wpk", name="wpk")
            wmask = sb.tile([128, NK * NR], bf16, tag="wmask", name="wmask")
            gw = sb.tile([NR, KH], bf16, tag="gw", name="gw")
            nc.sync.dma_start(uc[0][:], u_in[0:128, :])
            nc.sync.dma_start(wmask[:], wmask_in[:, :])
            nc.scalar.dma_start(wpk[:], wpack_in[:, :])
            nc.scalar.dma_start(uc[1][:], u_in[128:256, :])
            nc.scalar.dma_start(uc[3][:], u_in[384:512, :])
            nc.gpsimd.dma_start(uc[2][:], u_in[256:384, :])
            nc.gpsimd.dma_start(gw[:], gw_in[:, :])

            me2v = wpk[:, 0:256].rearrange("p (i r) -> p i r", i=2)
            vwtv = wpk[:, 256:768].rearrange("p (i c) -> p i c", i=2)

            Fcat = sb.tile([128, HW], bf16, tag="Fcat", name="Fcat")
            SAp = sb.tile([128, 4], f32, tag="SAp", name="SAp")
            SAb4 = sb.tile([128, 4], bf16, tag="SAb4", name="SAb4")
            rs = sb.tile([128, NK], f32, tag="rs", name="rs")
            rinv = sb.tile([128, NK], f32, tag="rinv", name="rinv")
            vtb = sb.tile([128, NK * C], bf16, tag="vtb", name="vtb")
            wsc = sb.tile([128, NK * NR], bf16, tag="wsc", name="wsc")
            Wsb = sb.tile([128, C], bf16, tag="Wsb", name="Wsb")
            osb0 = sb.tile([128, HW], bf16, tag="osb0", name="osb0")
            osb1 = sb.tile([128, HW], bf16, tag="osb1", name="osb1")

            # ---- phase 1: E2 + exp (all j), Vt (keys = cols 0:2048) ----
            for ci in range(4):
                ucv = uc[ci].rearrange("p (i x) -> p i x", i=2)
                e2 = ps.tile([128, 1024], f32, tag="big", bufs=2, name=f"e2_{ci}")
                for q in range(2):
                    nc.tensor.matmul(
                        e2[:, q * 512 : (q + 1) * 512], me2v,
                        ucv[:, :, q * 512 : (q + 1) * 512],
                        start=True, stop=True, perf_mode=DR,
                    )
                nc.scalar.activation(
                    Fcat[:, ci * 1024 : (ci + 1) * 1024], e2[:, 0:1024], AF.Exp,
                    scale=1.0 / SC, accum_out=SAp[:, ci : ci + 1],
                )
                if ci < 2:
                    for g in range(4):
                        vt = ps.tile([128, 512], f32, tag="vt", bufs=2,
                                     name=f"vt_{ci}_{g}")
                        for t2 in range(2):
                            kt = ci * 8 + g * 2 + t2
                            lx = (kt % 8) * 128
                            nc.tensor.matmul(
                                vt[:, t2 * 256 : (t2 + 1) * 256],
                                ucv[:, :, lx : lx + 128], vwtv,
                                start=True, stop=True, perf_mode=DR,
                            )
                        kt0 = ci * 8 + g * 2
                        nc.vector.tensor_copy(
                            vtb[:, kt0 * 256 : (kt0 + 2) * 256], vt[:]
                        )

            # ---- softmax denominators: SA -> gather -> 1/s ----
            nc.vector.reduce_sum(rs[:, 0:1], SAp[:, 0:4], axis=AX.X)
            nc.vector.tensor_copy(SAb4[:, 0:1], rs[:, 0:1])
            # dummy matmuls keep the PE's HAM clock gate warm through the
            # SA-reduce wait, so the W/P matmuls run at 2.4 GHz not 1.2
            wps = ps.tile([128, C], f32, tag="w", name="wps")
            for wd in range(8):
                nc.tensor.matmul(
                    wps[:], gw[:, 0:128], vtb[:, 0:C], start=True, stop=True
                )
            sg = ps.tile([128, NK], f32, tag="sg", name="sg")
            for kt in range(NK):
                nc.tensor.matmul(
                    sg[:, kt : kt + 1], gw[:, kt * 128 : (kt + 1) * 128],
                    SAb4[:, 0:1],
                    start=True, stop=True,
                )
            nc.vector.reciprocal(rinv[:], sg[:])

            # ---- W = (wmask/s)^T @ Vt ----
            for kt in range(NK):
                nc.vector.tensor_scalar(
                    wsc[:, kt * NR : (kt + 1) * NR],
                    wmask[:, kt * NR : (kt + 1) * NR],
                    rinv[:, kt : kt + 1], None, op0=ALU.mult,
                )
                nc.tensor.matmul(
                    wps[:], wsc[:, kt * NR : (kt + 1) * NR],
                    vtb[:, kt * C : (kt + 1) * C],
                    start=(kt == 0), stop=(kt == NK - 1),
                )
            nc.scalar.copy(Wsb[:], wps[:])

            # ---- P = W^T @ Fcat -> bf16 -> DRAM ----
            # 512-wide copies on both engines in parallel keep the PSUM
            # double-buffer from stalling the matmul stream.
            for ct in range(2):
                osb = osb0 if ct == 0 else osb1
                for ci in range(4):
                    idx = ct * 4 + ci
                    sl = slice(ci * 1024, (ci + 1) * 1024)
                    sl0 = slice(ci * 1024, ci * 1024 + 512)
                    sl1 = slice(ci * 1024 + 512, (ci + 1) * 1024)
                    if idx % 2 == 0:
                        pp = ps.tile([128, 1024], f32, tag="big", bufs=2,
                                     name=f"pp_{ct}_{ci}")
                        nc.tensor.matmul(
                            pp[:, 0:512],
                            Wsb[:, ct * 128 : (ct + 1) * 128], Fcat[:, sl0],
                            start=True, stop=True,
                        )
                        nc.tensor.matmul(
                            pp[:, 512:1024],
                            Wsb[:, ct * 128 : (ct + 1) * 128], Fcat[:, sl1],
                            start=True, stop=True,
                        )
                        nc.scalar.copy(osb[:, sl], pp[:])
                    else:
                        # odd chunks use the idle "vt" banks -> 3-deep PSUM
                        # pipelining, so matmuls never wait on the copies
                        ppa = ps.tile([128, 512], f32, tag="vt", bufs=2,
                                      name=f"ppa_{ct}_{ci}")
                        nc.tensor.matmul(
                            ppa[:], Wsb[:, ct * 128 : (ct + 1) * 128],
                            Fcat[:, sl0], start=True, stop=True,
                        )
                        ppb = ps.tile([128, 512], f32, tag="vt", bufs=2,
                                      name=f"ppb_{ct}_{ci}")
                        nc.tensor.matmul(
                            ppb[:], Wsb[:, ct * 128 : (ct + 1) * 128],
                            Fcat[:, sl1], start=True, stop=True,
                        )
                        nc.vector.tensor_copy(osb[:, sl0], ppa[:])
                        nc.vector.tensor_copy(osb[:, sl1], ppb[:])
                    qeng = nc.sync if ct == 0 else nc.gpsimd
                    qeng.dma_start(
                        out_d[ct * 128 : (ct + 1) * 128, sl], osb[:, sl]
                    )

    nc.compile()
    return nc


def _get_built():
    global _BUILT
    if _BUILT is None:
        _BUILT = _build()
    return _BUILT


def _prep_global(boundary_map, key_w1, bn_scale, bn_bias, bn_mean, bn_var,
                 key_w2, query_w, value_w):
    """Segment geometry + weight folding (all float64 host math)."""
    b = boundary_map.shape[0]
    h = 64
    H0 = boundary_map.shape[2]
    idx = (np.arange(h) * H0) // h
    bm = boundary_map[:, 0][:, idx][:, :, idx].reshape(b, HW).astype(np.float64)

    inv = bn_scale.astype(np.float64) / np.sqrt(bn_var.astype(np.float64) + 1e-5)
    beta = bn_bias.astype(np.float64) - bn_mean.astype(np.float64) * inv
    kw1f = key_w1[:, 0].astype(np.float64) * inv
    M = key_w2.astype(np.float64).T @ query_w.astype(np.float64)   # [64, 256]

    tall = bm.reshape(-1)
    lo, hi = tall.min(), tall.max()
    with np.errstate(divide="ignore", invalid="ignore"):
        tstar = np.where(np.abs(kw1f) > 1e-30, -beta / kw1f, np.inf)
    bps = np.sort(tstar[(tstar > lo) & (tstar < hi)])
    edges_true = np.concatenate([[lo], bps, [hi]])
    nseg = len(edges_true) - 1
    assert nseg <= S, f"too many relu segments: {nseg}"

    widths = np.diff(edges_true)
    cnt = np.histogram(tall, bins=edges_true)[0]
    score = widths * np.sqrt(cnt + 1)
    alloc = np.maximum(1, np.floor(score / score.sum() * S).astype(int))
    while alloc.sum() < S:
        alloc[np.argmax(score / alloc)] += 1
    while alloc.sum() > S:
        cand = np.where(alloc > 1)[0]
        alloc[cand[np.argmin((score / alloc)[cand])]] -= 1

    ps_lo = np.empty(S); ps_hi = np.empty(S); ps_true = np.empty(S, np.int64)
    si_out = 0
    for si in range(nseg):
        sub = np.linspace(edges_true[si], edges_true[si + 1], alloc[si] + 1)
        for k in range(alloc[si]):
            ps_lo[si_out] = sub[k]; ps_hi[si_out] = sub[k + 1]
            ps_true[si_out] = si
            si_out += 1
    assert si_out == S

    mids = 0.5 * (edges_true[:-1] + edges_true[1:])
    act = (kw1f[None, :] * mids[:, None] + beta[None, :]) > 0     # [nseg, 64]
    ME2 = np.zeros((NR, 64))
    for s in range(S):
        si = ps_true[s]
        for a, ta in enumerate((ps_lo[s], ps_hi[s])):
            ME2[2 * s + a] = ta * (act[si] * kw1f) + act[si] * beta
    me2m = ME2 @ M                                                # [128, 256]

    seg = np.clip(np.searchsorted(ps_hi, bm, side="left"), 0, S - 1)  # [b, HW]
    width = np.maximum(ps_hi[seg] - ps_lo[seg], 1e-12)
    wl = (ps_hi[seg] - bm) / width
    wh = 1.0 - wl
    return bm, me2m, seg, wl, wh


def _host_prep(boundary_map, uncertainty_map, key_w1, bn_scale, bn_bias,
               bn_mean, bn_var, key_w2, query_w, value_w):
    import ml_dtypes

    bf16 = ml_dtypes.bfloat16
    f8 = ml_dtypes.float8_e4m3
    b, c, h, w = uncertainty_map.shape
    _, me2m, seg, wl, wh = _prep_global(
        boundary_map, key_w1, bn_scale, bn_bias, bn_mean, bn_var,
        key_w2, query_w, value_w,
    )
    # wpack: me2-dr | vwt-dr, both pre-scaled by SC, channel-pair interleaved
    wpack = np.zeros((128, 768), np.float32)
    me2s = (me2m * SC).astype(np.float32)                          # [128r, 256c]
    vws = (value_w.T * SC).astype(np.float32)                      # [256c', 256c]
    for i in range(2):
        wpack[:, i * 128 : (i + 1) * 128] = me2s[:, i::2].T        # [p, r]
        wpack[:, 256 + i * 256 : 256 + (i + 1) * 256] = vws[i::2, :]
    wpack = wpack.astype(f8)

    in_maps = []
    for core in range(8):
        bi, kh = core // 2, core % 2
        u = uncertainty_map[bi].reshape(c, h * w).astype(np.float32)
        if kh == 1:
            u = np.concatenate([u[:, KH:], u[:, :KH]], axis=1)
        u8 = u.astype(f8)
        # chunk blocks: ub[ci*128+p, i*1024+x] = u8[2p+i, ci*1024+x]
        ub = np.ascontiguousarray(
            u8.reshape(128, 2, 4, 1024).transpose(2, 0, 1, 3).reshape(512, 2048)
        )
        ksl = slice(kh * KH, (kh + 1) * KH)
        sg_k = seg[bi, ksl]
        wmask = np.zeros((KH, NR), np.float32)
        kk = np.arange(KH)
        wmask[kk, 2 * sg_k] = wl[bi, ksl]
        wmask[kk, 2 * sg_k + 1] = wh[bi, ksl]
        wm_dev = np.ascontiguousarray(
            wmask.reshape(NK, 128, NR).transpose(1, 0, 2).reshape(128, NK * NR)
        ).astype(bf16)
        gw_dev = np.ascontiguousarray(
            wmask.reshape(NK, 128, NR).transpose(2, 0, 1).reshape(NR, KH)
        ).astype(bf16)
        in_maps.append({
            "u_in": ub,
            "wpack_in": wpack,
            "wmask_in": wm_dev,
            "gw_in": gw_dev,
        })
    return in_maps


def kernel(boundary_map, uncertainty_map, key_w1, bn_scale, bn_bias,
           bn_mean, bn_var, key_w2, query_w, value_w, gamma):
    global LAST_RESULTS
    from concourse.bass_utils import run_bass_kernel_spmd

    nc = _get_built()
    in_maps = _host_prep(
        np.asarray(boundary_map), np.asarray(uncertainty_map), np.asarray(key_w1),
        np.asarray(bn_scale), np.asarray(bn_bias), np.asarray(bn_mean),
        np.asarray(bn_var), np.asarray(key_w2), np.asarray(query_w),
        np.asarray(value_w),
    )
    kwargs = {}
    if TRACE:
        kwargs["trace"] = True
        if TRACE_CORES is not None:
            kwargs["trace_cores"] = TRACE_CORES
    res = run_bass_kernel_spmd(nc, in_maps, core_ids=list(range(8)), **kwargs)
    LAST_RESULTS = res

    b, c, h, w = uncertainty_map.shape
    g = np.float32(np.asarray(gamma).reshape(-1)[0] / SC)
    out = np.empty((b, c, h * w), np.float32)
    um = np.asarray(uncertainty_map)
    for bi in range(b):
        P0 = res.results[2 * bi]["outp"].astype(np.float32)
        P1 = res.results[2 * bi + 1]["outp"].astype(np.float32)
        P1 = np.concatenate([P1[:, KH:], P1[:, :KH]], axis=1)
        out[bi] = g * (P0 + P1) + um[bi].reshape(c, h * w)
    return out.reshape(b, c, h, w)
